# revision 43
# baseline (speedup 1.0000x reference)
"""Block-sparse attention TRN2 kernel (8 NeuronCores, SPMD over batch*heads).

Contract: kernel(**inputs) takes FULL unsharded inputs
  query/key/value: (2, 16, 2048, 128) f32, block_mask: (16, 16) bool,
  block_size: 128
and returns the FULL (2, 16, 2048, 128) f32 output.

Math per (b, h): for each 128x128 block pair (i, j) with block_mask[i, j]:
  A_ij = softmax(Q_i K_j^T / sqrt(128)) (softmax per block row, no
  cross-block merge), O_i = sum_j A_ij V_j.

Device layout ([k, q] orientation so no on-chip transposes are needed):
  For key block j, scores for the active query blocks are packed into
  512-col (one PSUM bank) chunks: S^T = matmul(lhsT=KT[:, j], rhs=QT runs)
  in f16 (1 cyc/row at any width; scores accumulate fp32 in PSUM). exp on
  ACT (PSUM f32 -> SBUF f16). Denominators = column sums via
  matmul(lhsT=ones[128,128]), replicated across partitions in PSUM.
  reciprocal_approx_fast with f16 OUTPUT (direct _custom_dve call; the
  public wrapper only allows f32 out) so the normalize multiply runs
  f16xf16 at DVE 2x_1P rate; multiplies alternate DVE / GPSIMD to balance
  the two engines (DVE also owns every reciprocal - GPSIMD has no PSUM
  port). O^T += V_j^T.T @ Ahat^T accumulates in PSUM per 512-col output
  bank. Each head runs as two half-schedules (query rows 0-7 / 8-15) so
  only 2 output banks are live at a time, freeing PSUM for a 4-deep score
  pipeline (s 4 + d 2 + o 2 = 8 banks). Emission is software-pipelined
  (MM1/exp at p, denom/recip/mult at p-1, MM2 at p-3) so the strict-FIFO
  engine queues never head-of-line block on cross-engine dependencies.
  All heads' inputs prefetch up front; outputs drain as f16 (ACT copy +
  DMA) at the end of each half. Q^T/K^T/V packing and the final
  O^T -> O transpose happen on the host.
"""

import math

import numpy as np

B, H, S, D = 2, 16, 2048, 128
BS = 128
NB = S // BS
N_CORES = 8
N_HEADS = B * H
HPC = N_HEADS // N_CORES  # heads per core
CH = 512  # chunk columns = one PSUM bank of f32
SCALE = 1.0 / math.sqrt(float(D))

# Per-pair engine for the normalize multiply: 'D' = DVE, 'G' = GPSIMD.
# Tuned so DVE (which also owns every reciprocal) and GPSIMD balance.
MULT_PATTERN = "DGG"
# Per-bank engine for the output drain copy: 'A' = ACT, 'V' = DVE.
DRAIN_PATTERN = "AAAA"


def _plan(mask, rows):
    """Mask-derived emission plan for query rows `rows` (one output half).

    Returns a flat chunk schedule; each chunk is (used, mm1s, pieces) with
      mm1s   = (off_in_chunk, [qoff, ...], width, j); two qoffs means a
               paired single-block matmul via a 3-level access pattern.
      pieces = (q_out_col, width, off_in_chunk, j) MM2 pieces, split at
               output PSUM bank boundaries and first-touch flips.
    Partial tail chunks are merged ACROSS key blocks j (exp/denominator/
    normalize are j-agnostic; all KT/V slices are SBUF-resident), which
    cuts per-chunk op overheads on ACT/DVE by ~20%.
    """
    mask = np.asarray(mask).astype(bool)
    assert mask.shape == (NB, NB)
    rows = set(rows)
    cap = CH // BS  # blocks per chunk

    # Per-j FFD bin packing into <=cap-block bins.
    groups = []  # (j, [(i0, ln), ...]) per finalized bin, emission order
    pending = []  # [(j, item)] accumulating partial tails
    pend_fill = 0

    def flush():
        nonlocal pend_fill
        if pending:
            groups.append(list(pending))
            pending.clear()
            pend_fill = 0

    for j in range(NB):
        act = [i for i in range(NB) if i in rows and mask[i, j]]
        runs = []
        for i in act:
            if runs and runs[-1][0] + runs[-1][1] == i:
                runs[-1][1] += 1
            else:
                runs.append([i, 1])
        items = []
        for i0, ln in runs:
            while ln > cap:
                items.append((i0, cap))
                i0 += cap
                ln -= cap
            items.append((i0, ln))
        bins = []
        for i0, ln in sorted(items, key=lambda x: -x[1]):
            for b in bins:
                if b[0] + ln <= cap:
                    b[0] += ln
                    b[1].append((i0, ln))
                    break
            else:
                bins.append([ln, [(i0, ln)]])
        for fill, bitems in bins:
            if fill == cap:
                groups.append([(j, it) for it in sorted(bitems)])
            else:
                if pend_fill + fill > cap:
                    flush()
                pending.extend((j, it) for it in sorted(bitems))
                pend_fill += fill
    flush()

    # Full 512-col chunks first, partial tails last, so chunk PAIRS have at
    # most one interior gap (which the build memsets once).
    groups.sort(key=lambda g: -sum(it[1] for _, it in g))

    # Lay out each chunk and derive matmul descriptors + output pieces in
    # emission order (first-touch of an output block = overwrite; later
    # touches accumulate; a single matmul must be uniformly one or the
    # other and may not straddle an output bank).
    sched = []
    bank_counts = [0] * (S // CH)
    touched = set()
    for gitems in groups:
        byj = {}
        for j, it in gitems:
            byj.setdefault(j, []).append(it)
        placed = []  # (off, qoff, w, j)
        mm1s = []
        off = 0
        for j in sorted(byj):
            jitems = byj[j]
            longs = sorted([it for it in jitems if it[1] > 1])
            singles = sorted([it for it in jitems if it[1] == 1])
            sing_offs = []
            for i0, ln in longs + singles:
                placed.append((off, i0 * BS, ln * BS, j))
                if ln > 1:
                    mm1s.append((off, [i0 * BS], ln * BS, j))
                else:
                    sing_offs.append((off, i0 * BS))
                off += ln * BS
            for k in range(0, len(sing_offs) - 1, 2):
                mm1s.append(
                    (sing_offs[k][0], [sing_offs[k][1], sing_offs[k + 1][1]],
                     2 * BS, j)
                )
            if len(sing_offs) % 2:
                mm1s.append((sing_offs[-1][0], [sing_offs[-1][1]], BS, j))
        used = off
        pieces = []
        for o, qoff, w, j in placed:
            ib0 = qoff // BS
            nblk = w // BS
            blk = 0
            while blk < nblk:
                ib = ib0 + blk
                ft = ib not in touched
                bank = (ib * BS) // CH
                end = blk + 1
                while end < nblk:
                    ib2 = ib0 + end
                    if (ib2 not in touched) != ft or (ib2 * BS) // CH != bank:
                        break
                    end += 1
                for b2 in range(blk, end):
                    touched.add(ib0 + b2)
                qo = ib * BS
                wp = (end - blk) * BS
                pieces.append((qo, wp, o + (qo - qoff), j))
                bank_counts[bank] += 1
                blk = end
        sched.append((used, mm1s, pieces))
    empty_rows = [i for i in rows if not mask[i].any()]
    return sched, bank_counts, empty_rows


def _build(mask):
    import concourse.bass as bass
    import concourse.bacc as bacc
    import concourse.tile as tile
    from concourse import mybir
    from concourse.dve_ops import RECIP_APPROX_FAST_CONSTS, RECIPROCAL_APPROX_FAST

    f32 = mybir.dt.float32
    f16 = mybir.dt.float16
    AF = mybir.ActivationFunctionType
    RC = RECIP_APPROX_FAST_CONSTS

    # Each head runs as two half-schedules (query rows 0-7, then 8-15) so the
    # output accumulator needs only 2 PSUM banks at a time, freeing banks for
    # a deeper (bufs=3) score-pair pipeline.
    n_banks = S // CH
    rows_per_half = NB // 2
    halves = [
        _plan(mask, range(hf * rows_per_half, (hf + 1) * rows_per_half))
        for hf in range(2)
    ]

    nc = bacc.Bacc(
        "TRN2",
        target_bir_lowering=False,
        debug=False,
        enable_asserts=False,
        num_devices=N_CORES,
    )
    qt_d = nc.dram_tensor("qt", (HPC, D, S), f16, kind="ExternalInput").ap()
    kt_d = nc.dram_tensor("kt", (HPC, D, S), f16, kind="ExternalInput").ap()
    v_d = nc.dram_tensor("v", (HPC, BS, NB * BS), f16, kind="ExternalInput").ap()
    ot_d = nc.dram_tensor("ot", (HPC, D, S), f16, kind="ExternalOutput").ap()

    with tile.TileContext(nc) as tc:
        with (
            tc.tile_pool(name="heads", bufs=4) as heads,
            tc.tile_pool(name="const", bufs=1) as const,
            tc.tile_pool(name="e", bufs=6) as epool,
            tc.tile_pool(name="eh", bufs=6) as ehpool,
            tc.tile_pool(name="r", bufs=6) as rpool,
            tc.tile_pool(name="ps_d", bufs=2, space="PSUM") as ps_d,
            tc.tile_pool(name="outp", bufs=8) as outpool,
            # Split-half output (2 banks live) frees PSUM for a 4-deep
            # score pipeline: s 4 + d 2 + o 2 = 8 banks.
            tc.tile_pool(name="ps_s", bufs=4, space="PSUM") as ps_s,
            tc.tile_pool(name="ps_o", bufs=1, space="PSUM") as ps_o,
        ):
            ones_t = const.tile([BS, BS], f16)
            nc.vector.memset(ones_t[:], 1.0)

            # Build the flat pair-record list (one record per chunk pair).
            recs = []
            for h_hf in range(HPC * 2):
                h, hf = divmod(h_hf, 2)
                sched, bank_counts, empty_rows = halves[hf]
                for t0 in range(len(sched)):
                    recs.append(
                        {
                            "h": h,
                            "hf": hf,
                            "chunk": sched[t0],
                            "first": t0 == 0,
                            "last": t0 + 1 >= len(sched),
                            "bank_counts": bank_counts,
                            "empty_rows": empty_rows,
                        }
                    )

            # Stage emitters. Emission order is software-pipelined so the PE
            # FIFO never head-of-line blocks: iteration p emits MM1s(p) /
            # exp(p), then denominators(p-1) / recip(p-1) / mult(p-1), then
            # MM2s(p-2) -- every PE instruction's dependencies are ~2 stages
            # old by the time the engine reaches it.
            head_tiles = {}
            half_state = {}
            # Input prefetch for all heads. Descriptor generation costs
            # ~0.65us per dma_start on the issuing engine, so the first
            # head's loads are spread across four engines to overlap the
            # generation, and later heads go on sync (drains come much
            # later, so sync has slack).
            for h in range(HPC):
                qt_t = heads.tile([D, S], f16, tag="qt", name="qt_t")
                kt_t = heads.tile([D, S], f16, tag="kt", name="kt_t")
                v_t = heads.tile([BS, NB * BS], f16, tag="v", name="v_t")
                if h == 0:
                    for q4 in range(4):
                        sl = slice(q4 * S // 4, (q4 + 1) * S // 4)
                        nc.sync.dma_start(out=kt_t[:, sl], in_=kt_d[h, :, sl])
                    for q2 in range(2):
                        sl = slice(q2 * S // 2, (q2 + 1) * S // 2)
                        nc.scalar.dma_start(out=qt_t[:, sl], in_=qt_d[h, :, sl])
                    nc.gpsimd.dma_start(out=v_t[:], in_=v_d[h])
                else:
                    nc.sync.dma_start(out=kt_t[:], in_=kt_d[h])
                    nc.sync.dma_start(out=qt_t[:], in_=qt_d[h])
                    nc.sync.dma_start(out=v_t[:], in_=v_d[h])
                head_tiles[h] = (qt_t, kt_t, v_t)

            def s0_mm1_exp(p, rec):
                h, hf = rec["h"], rec["hf"]
                qt_t, kt_t, v_t = head_tiles[h]
                if rec["first"]:
                    hbank = hf * (n_banks // 2)
                    o_tiles = {
                        b: ps_o.tile([D, CH], f32, name=f"ob{b % 2}", tag=f"ot{b % 2}")
                        for b in range(hbank, hbank + n_banks // 2)
                    }
                    for i in rec["empty_rows"]:
                        b, c = divmod(i * BS, CH)
                        nc.vector.memset(o_tiles[b][:, c : c + BS], 0.0)
                    half_state[(h, hf)] = {
                        "o_tiles": o_tiles,
                        "remaining": list(rec["bank_counts"]),
                        "started": set(),
                    }
                used, mm1s, pieces = rec["chunk"]
                s_ps = ps_s.tile([BS, CH], f32, name="s_ps")
                for idx, (off, qoffs, w, j) in enumerate(mm1s):
                    if len(qoffs) == 2:
                        base = qt_t[:, qoffs[0] : qoffs[0] + BS]
                        rhs = bass.AP(
                            tensor=base.tensor,
                            offset=base.offset,
                            ap=[
                                base.ap[0],
                                [qoffs[1] - qoffs[0], 2],
                                [1, BS],
                            ],
                        )
                    else:
                        rhs = qt_t[:, qoffs[0] : qoffs[0] + w]
                    nc.tensor.matmul(
                        s_ps[:, off : off + w],
                        lhsT=kt_t[:, j * BS : (j + 1) * BS],
                        rhs=rhs,
                        start=(idx == 0),
                        stop=(idx == len(mm1s) - 1),
                    )
                e_t = epool.tile([BS, CH], f16, name="e_t")
                nc.scalar.activation(
                    e_t[:, :used], s_ps[:, :used], AF.Exp, scale=SCALE
                )
                rec["s_ps"], rec["e_t"] = s_ps, e_t

            def s2_denom_norm(p, rec):
                e_t = rec["e_t"]
                used = rec["chunk"][0]
                d_ps = ps_d.tile([BS, CH], f32, name="d_ps")
                nc.tensor.matmul(
                    d_ps[:, :used],
                    lhsT=ones_t[:],
                    rhs=e_t[:, :used],
                    start=True,
                    stop=True,
                )
                # reciprocal_approx_fast with f16 output (direct custom-DVE
                # call; ~51 ULP in f32, then f16 rounding) so the multiply
                # below gets DVE 2x_1P rate.
                r_t = rpool.tile([BS, CH], f16, name="r_t")
                nc.vector._custom_dve(
                    RECIPROCAL_APPROX_FAST,
                    out=r_t[:, :used],
                    in0=d_ps[:, :used],
                    s0=RC["s0"],
                    s1=RC["s1"],
                    imm2=RC["imm2"],
                )
                eh_t = ehpool.tile([BS, CH], f16, name="eh_t")
                mult_on = MULT_PATTERN[p % len(MULT_PATTERN)]
                mult_eng = nc.gpsimd if mult_on == "G" else nc.vector
                mult_eng.tensor_tensor(
                    out=eh_t[:, :used],
                    in0=e_t[:, :used],
                    in1=r_t[:, :used],
                    op=mybir.AluOpType.mult,
                )
                rec["eh_t"] = eh_t

            def s5_mm2_drain(p, rec):
                h, hf = rec["h"], rec["hf"]
                st = half_state[(h, hf)]
                o_tiles, remaining, started = (
                    st["o_tiles"],
                    st["remaining"],
                    st["started"],
                )
                _, _, v_t = head_tiles[h]
                eh_t = rec["eh_t"]
                for qo, wp, op, j in rec["chunk"][2]:
                    b = qo // CH
                    first = b not in started
                    started.add(b)
                    remaining[b] -= 1
                    nc.tensor.matmul(
                        o_tiles[b][:, qo - b * CH : qo - b * CH + wp],
                        lhsT=v_t[:, j * BS : (j + 1) * BS],
                        rhs=eh_t[:, op : op + wp],
                        start=first,
                        stop=(remaining[b] == 0),
                    )
                if rec["last"]:
                    for b in sorted(o_tiles):
                        o_sb = outpool.tile([D, CH], f16, tag="osb", name="o_sb")
                        if DRAIN_PATTERN[b % len(DRAIN_PATTERN)] == "A":
                            nc.scalar.copy(o_sb[:], o_tiles[b][:])
                        else:
                            nc.vector.tensor_copy(out=o_sb[:], in_=o_tiles[b][:])
                        nc.sync.dma_start(
                            out=ot_d[h, :, b * CH : (b + 1) * CH], in_=o_sb[:]
                        )

            for p in range(len(recs) + 3):
                if p < len(recs):
                    s0_mm1_exp(p, recs[p])
                if 1 <= p <= len(recs):
                    s2_denom_norm(p - 1, recs[p - 1])
                if p >= 3:
                    s5_mm2_drain(p - 3, recs[p - 3])

    nc.finalize()
    return nc


_CACHE = {}


def _get_program(mask):
    key = np.asarray(mask).astype(bool).tobytes()
    if key not in _CACHE:
        _CACHE[key] = _build(mask)
    return _CACHE[key]


def _shard_inputs(query, key, value):
    q = np.ascontiguousarray(query, dtype=np.float32).reshape(N_HEADS, S, D)
    k = np.ascontiguousarray(key, dtype=np.float32).reshape(N_HEADS, S, D)
    v = np.ascontiguousarray(value, dtype=np.float32).reshape(N_HEADS, S, D)
    qt = np.ascontiguousarray(q.transpose(0, 2, 1).astype(np.float16))  # (32, D, S)
    kt = np.ascontiguousarray(k.transpose(0, 2, 1).astype(np.float16))
    v16 = np.ascontiguousarray(
        v.reshape(N_HEADS, NB, BS, D).transpose(0, 2, 1, 3).astype(np.float16)
    ).reshape(N_HEADS, BS, NB * BS)
    in_maps = []
    for c in range(N_CORES):
        sl = slice(c * HPC, (c + 1) * HPC)
        in_maps.append(
            {
                "qt": np.ascontiguousarray(qt[sl]),
                "kt": np.ascontiguousarray(kt[sl]),
                "v": np.ascontiguousarray(v16[sl]),
            }
        )
    return in_maps


def _unshard_output(results):
    ot = np.concatenate([r["ot"] for r in results], axis=0)  # (32, D, S)
    out = ot.transpose(0, 2, 1).reshape(B, H, S, D)
    return np.ascontiguousarray(out, dtype=np.float32)


def kernel(query, key, value, block_mask, block_size, _trace=False):
    from concourse.bass_utils import run_bass_kernel_spmd

    assert int(block_size) == BS
    nc = _get_program(block_mask)
    in_maps = _shard_inputs(query, key, value)
    res = run_bass_kernel_spmd(nc, in_maps, core_ids=list(range(N_CORES)), trace=_trace)
    out = _unshard_output(res.results)
    if _trace:
        return out, res
    return out


# revision 44
# speedup vs baseline: 1.0000x; 1.0000x over previous
"""Block-sparse attention TRN2 kernel (8 NeuronCores, SPMD over batch*heads).

Contract: kernel(**inputs) takes FULL unsharded inputs
  query/key/value: (2, 16, 2048, 128) f32, block_mask: (16, 16) bool,
  block_size: 128
and returns the FULL (2, 16, 2048, 128) f32 output.

Math per (b, h): for each 128x128 block pair (i, j) with block_mask[i, j]:
  A_ij = softmax(Q_i K_j^T / sqrt(128)) (softmax per block row, no
  cross-block merge), O_i = sum_j A_ij V_j.

Device layout ([k, q] orientation so no on-chip transposes are needed):
  For key block j, scores for the active query blocks are packed into
  512-col (one PSUM bank) chunks: S^T = matmul(lhsT=KT[:, j], rhs=QT runs)
  in f16 (1 cyc/row at any width; scores accumulate fp32 in PSUM). exp on
  ACT (PSUM f32 -> SBUF f16). Denominators = column sums via
  matmul(lhsT=ones[128,128]), replicated across partitions in PSUM.
  reciprocal_approx_fast with f16 OUTPUT (direct _custom_dve call; the
  public wrapper only allows f32 out) so the normalize multiply runs
  f16xf16 at DVE 2x_1P rate; multiplies alternate DVE / GPSIMD to balance
  the two engines (DVE also owns every reciprocal - GPSIMD has no PSUM
  port). O^T += V_j^T.T @ Ahat^T accumulates in PSUM per 512-col output
  bank. Each head runs as two half-schedules (query rows 0-7 / 8-15) so
  only 2 output banks are live at a time, freeing PSUM for a 4-deep score
  pipeline (s 4 + d 2 + o 2 = 8 banks). Emission is software-pipelined
  (MM1/exp at p, denom/recip/mult at p-1, MM2 at p-3) so the strict-FIFO
  engine queues never head-of-line block on cross-engine dependencies.
  All heads' inputs prefetch up front; outputs drain as f16 (ACT copy +
  DMA) at the end of each half. Q^T/K^T/V packing and the final
  O^T -> O transpose happen on the host.
"""

import math

import numpy as np

B, H, S, D = 2, 16, 2048, 128
BS = 128
NB = S // BS
N_CORES = 8
N_HEADS = B * H
HPC = N_HEADS // N_CORES  # heads per core
CH = 512  # chunk columns = one PSUM bank of f32
SCALE = 1.0 / math.sqrt(float(D))

# Per-pair engine for the normalize multiply: 'D' = DVE, 'G' = GPSIMD.
# Tuned so DVE (which also owns every reciprocal) and GPSIMD balance.
MULT_PATTERN = "DGG"
# Per-bank engine for the output drain copy: 'A' = ACT, 'V' = DVE.
DRAIN_PATTERN = "AAAA"


def _plan(mask, rows):
    """Mask-derived emission plan for query rows `rows` (one output half).

    Returns a flat chunk schedule; each chunk is (used, mm1s, pieces) with
      mm1s   = (off_in_chunk, [qoff, ...], width, j); two qoffs means a
               paired single-block matmul via a 3-level access pattern.
      pieces = (q_out_col, width, off_in_chunk, j) MM2 pieces, split at
               output PSUM bank boundaries and first-touch flips.
    Partial tail chunks are merged ACROSS key blocks j (exp/denominator/
    normalize are j-agnostic; all KT/V slices are SBUF-resident), which
    cuts per-chunk op overheads on ACT/DVE by ~20%.
    """
    mask = np.asarray(mask).astype(bool)
    assert mask.shape == (NB, NB)
    rows = set(rows)
    cap = CH // BS  # blocks per chunk

    # Per-j FFD bin packing into <=cap-block bins.
    groups = []  # (j, [(i0, ln), ...]) per finalized bin, emission order
    pending = []  # [(j, item)] accumulating partial tails
    pend_fill = 0

    def flush():
        nonlocal pend_fill
        if pending:
            groups.append(list(pending))
            pending.clear()
            pend_fill = 0

    for j in range(NB):
        act = [i for i in range(NB) if i in rows and mask[i, j]]
        runs = []
        for i in act:
            if runs and runs[-1][0] + runs[-1][1] == i:
                runs[-1][1] += 1
            else:
                runs.append([i, 1])
        items = []
        for i0, ln in runs:
            while ln > cap:
                items.append((i0, cap))
                i0 += cap
                ln -= cap
            items.append((i0, ln))
        bins = []
        for i0, ln in sorted(items, key=lambda x: -x[1]):
            for b in bins:
                if b[0] + ln <= cap:
                    b[0] += ln
                    b[1].append((i0, ln))
                    break
            else:
                bins.append([ln, [(i0, ln)]])
        for fill, bitems in bins:
            if fill == cap:
                groups.append([(j, it) for it in sorted(bitems)])
            else:
                if pend_fill + fill > cap:
                    flush()
                pending.extend((j, it) for it in sorted(bitems))
                pend_fill += fill
    flush()

    # Full 512-col chunks first, partial tails last, so chunk PAIRS have at
    # most one interior gap (which the build memsets once).
    groups.sort(key=lambda g: -sum(it[1] for _, it in g))

    # Lay out each chunk and derive matmul descriptors + output pieces in
    # emission order (first-touch of an output block = overwrite; later
    # touches accumulate; a single matmul must be uniformly one or the
    # other and may not straddle an output bank).
    sched = []
    bank_counts = [0] * (S // CH)
    touched = set()
    for gitems in groups:
        byj = {}
        for j, it in gitems:
            byj.setdefault(j, []).append(it)
        placed = []  # (off, qoff, w, j)
        mm1s = []
        off = 0
        for j in sorted(byj):
            jitems = byj[j]
            longs = sorted([it for it in jitems if it[1] > 1])
            singles = sorted([it for it in jitems if it[1] == 1])
            sing_offs = []
            for i0, ln in longs + singles:
                placed.append((off, i0 * BS, ln * BS, j))
                if ln > 1:
                    mm1s.append((off, [i0 * BS], ln * BS, j))
                else:
                    sing_offs.append((off, i0 * BS))
                off += ln * BS
            for k in range(0, len(sing_offs) - 1, 2):
                mm1s.append(
                    (sing_offs[k][0], [sing_offs[k][1], sing_offs[k + 1][1]],
                     2 * BS, j)
                )
            if len(sing_offs) % 2:
                mm1s.append((sing_offs[-1][0], [sing_offs[-1][1]], BS, j))
        used = off
        pieces = []
        for o, qoff, w, j in placed:
            ib0 = qoff // BS
            nblk = w // BS
            blk = 0
            while blk < nblk:
                ib = ib0 + blk
                ft = ib not in touched
                bank = (ib * BS) // CH
                end = blk + 1
                while end < nblk:
                    ib2 = ib0 + end
                    if (ib2 not in touched) != ft or (ib2 * BS) // CH != bank:
                        break
                    end += 1
                for b2 in range(blk, end):
                    touched.add(ib0 + b2)
                qo = ib * BS
                wp = (end - blk) * BS
                pieces.append((qo, wp, o + (qo - qoff), j))
                bank_counts[bank] += 1
                blk = end
        sched.append((used, mm1s, pieces))
    empty_rows = [i for i in rows if not mask[i].any()]
    return sched, bank_counts, empty_rows


def _build(mask):
    import concourse.bass as bass
    import concourse.bacc as bacc
    import concourse.tile as tile
    from concourse import mybir
    from concourse.dve_ops import RECIP_APPROX_FAST_CONSTS, RECIPROCAL_APPROX_FAST

    f32 = mybir.dt.float32
    f16 = mybir.dt.float16
    AF = mybir.ActivationFunctionType
    RC = RECIP_APPROX_FAST_CONSTS

    # Each head runs as two half-schedules (query rows 0-7, then 8-15) so the
    # output accumulator needs only 2 PSUM banks at a time, freeing banks for
    # a deeper (bufs=3) score-pair pipeline.
    n_banks = S // CH
    rows_per_half = NB // 2
    halves = [
        _plan(mask, range(hf * rows_per_half, (hf + 1) * rows_per_half))
        for hf in range(2)
    ]

    nc = bacc.Bacc(
        "TRN2",
        target_bir_lowering=False,
        debug=False,
        enable_asserts=False,
        num_devices=N_CORES,
    )
    qt_d = nc.dram_tensor("qt", (HPC, D, S), f16, kind="ExternalInput").ap()
    kt_d = nc.dram_tensor("kt", (HPC, D, S), f16, kind="ExternalInput").ap()
    v_d = nc.dram_tensor("v", (HPC, BS, NB * BS), f16, kind="ExternalInput").ap()
    ot_d = nc.dram_tensor("ot", (HPC, D, S), f16, kind="ExternalOutput").ap()

    with tile.TileContext(nc) as tc:
        with (
            tc.tile_pool(name="heads", bufs=4) as heads,
            tc.tile_pool(name="const", bufs=1) as const,
            tc.tile_pool(name="e", bufs=6) as epool,
            tc.tile_pool(name="eh", bufs=6) as ehpool,
            tc.tile_pool(name="r", bufs=6) as rpool,
            tc.tile_pool(name="ps_d", bufs=2, space="PSUM") as ps_d,
            tc.tile_pool(name="outp", bufs=8) as outpool,
            # Split-half output (2 banks live) frees PSUM for a 4-deep
            # score pipeline: s 4 + d 2 + o 2 = 8 banks.
            tc.tile_pool(name="ps_s", bufs=4, space="PSUM") as ps_s,
            tc.tile_pool(name="ps_o", bufs=1, space="PSUM") as ps_o,
        ):
            ones_t = const.tile([BS, BS], f16)
            nc.vector.memset(ones_t[:], 1.0)

            # Build the flat pair-record list (one record per chunk pair).
            recs = []
            for h_hf in range(HPC * 2):
                h, hf = divmod(h_hf, 2)
                sched, bank_counts, empty_rows = halves[hf]
                for t0 in range(len(sched)):
                    recs.append(
                        {
                            "h": h,
                            "hf": hf,
                            "chunk": sched[t0],
                            "first": t0 == 0,
                            "last": t0 + 1 >= len(sched),
                            "bank_counts": bank_counts,
                            "empty_rows": empty_rows,
                        }
                    )

            # Stage emitters. Emission order is software-pipelined so the PE
            # FIFO never head-of-line blocks: iteration p emits MM1s(p) /
            # exp(p), then denominators(p-1) / recip(p-1) / mult(p-1), then
            # MM2s(p-2) -- every PE instruction's dependencies are ~2 stages
            # old by the time the engine reaches it.
            head_tiles = {}
            half_state = {}
            # Input prefetch for all heads. Descriptor generation costs
            # ~0.65us per dma_start on the issuing engine, so the first
            # head's loads are spread across four engines to overlap the
            # generation, and later heads go on sync (drains come much
            # later, so sync has slack).
            for h in range(HPC):
                qt_t = heads.tile([D, S], f16, tag="qt", name="qt_t")
                kt_t = heads.tile([D, S], f16, tag="kt", name="kt_t")
                v_t = heads.tile([BS, NB * BS], f16, tag="v", name="v_t")
                if h == 0:
                    for q4 in range(4):
                        sl = slice(q4 * S // 4, (q4 + 1) * S // 4)
                        nc.sync.dma_start(out=kt_t[:, sl], in_=kt_d[h, :, sl])
                    for q2 in range(2):
                        sl = slice(q2 * S // 2, (q2 + 1) * S // 2)
                        nc.scalar.dma_start(out=qt_t[:, sl], in_=qt_d[h, :, sl])
                    nc.gpsimd.dma_start(out=v_t[:], in_=v_d[h])
                else:
                    nc.sync.dma_start(out=kt_t[:], in_=kt_d[h])
                    nc.sync.dma_start(out=qt_t[:], in_=qt_d[h])
                    nc.sync.dma_start(out=v_t[:], in_=v_d[h])
                head_tiles[h] = (qt_t, kt_t, v_t)

            def s0_mm1_exp(p, rec):
                h, hf = rec["h"], rec["hf"]
                qt_t, kt_t, v_t = head_tiles[h]
                if rec["first"]:
                    hbank = hf * (n_banks // 2)
                    o_tiles = {
                        b: ps_o.tile([D, CH], f32, name=f"ob{b % 2}", tag=f"ot{b % 2}")
                        for b in range(hbank, hbank + n_banks // 2)
                    }
                    for i in rec["empty_rows"]:
                        b, c = divmod(i * BS, CH)
                        nc.vector.memset(o_tiles[b][:, c : c + BS], 0.0)
                    half_state[(h, hf)] = {
                        "o_tiles": o_tiles,
                        "remaining": list(rec["bank_counts"]),
                        "started": set(),
                    }
                used, mm1s, pieces = rec["chunk"]
                s_ps = ps_s.tile([BS, CH], f32, name="s_ps")
                for idx, (off, qoffs, w, j) in enumerate(mm1s):
                    if len(qoffs) == 2:
                        base = qt_t[:, qoffs[0] : qoffs[0] + BS]
                        rhs = bass.AP(
                            tensor=base.tensor,
                            offset=base.offset,
                            ap=[
                                base.ap[0],
                                [qoffs[1] - qoffs[0], 2],
                                [1, BS],
                            ],
                        )
                    else:
                        rhs = qt_t[:, qoffs[0] : qoffs[0] + w]
                    nc.tensor.matmul(
                        s_ps[:, off : off + w],
                        lhsT=kt_t[:, j * BS : (j + 1) * BS],
                        rhs=rhs,
                        start=(idx == 0),
                        stop=(idx == len(mm1s) - 1),
                    )
                e_t = epool.tile([BS, CH], f16, name="e_t")
                nc.scalar.activation(
                    e_t[:, :used], s_ps[:, :used], AF.Exp, scale=SCALE
                )
                rec["s_ps"], rec["e_t"] = s_ps, e_t

            def s2_denom_norm(p, rec):
                e_t = rec["e_t"]
                used = rec["chunk"][0]
                d_ps = ps_d.tile([BS, CH], f32, name="d_ps")
                nc.tensor.matmul(
                    d_ps[:, :used],
                    lhsT=ones_t[:],
                    rhs=e_t[:, :used],
                    start=True,
                    stop=True,
                )
                # reciprocal_approx_fast with f16 output (direct custom-DVE
                # call; ~51 ULP in f32, then f16 rounding) so the multiply
                # below gets DVE 2x_1P rate.
                r_t = rpool.tile([BS, CH], f16, name="r_t")
                nc.vector._custom_dve(
                    RECIPROCAL_APPROX_FAST,
                    out=r_t[:, :used],
                    in0=d_ps[:, :used],
                    s0=RC["s0"],
                    s1=RC["s1"],
                    imm2=RC["imm2"],
                )
                eh_t = ehpool.tile([BS, CH], f16, name="eh_t")
                mult_on = MULT_PATTERN[p % len(MULT_PATTERN)]
                mult_eng = nc.gpsimd if mult_on == "G" else nc.vector
                mult_eng.tensor_tensor(
                    out=eh_t[:, :used],
                    in0=e_t[:, :used],
                    in1=r_t[:, :used],
                    op=mybir.AluOpType.mult,
                )
                rec["eh_t"] = eh_t

            def s5_mm2_drain(p, rec):
                h, hf = rec["h"], rec["hf"]
                st = half_state[(h, hf)]
                o_tiles, remaining, started = (
                    st["o_tiles"],
                    st["remaining"],
                    st["started"],
                )
                _, _, v_t = head_tiles[h]
                eh_t = rec["eh_t"]
                for qo, wp, op, j in rec["chunk"][2]:
                    b = qo // CH
                    first = b not in started
                    started.add(b)
                    remaining[b] -= 1
                    nc.tensor.matmul(
                        o_tiles[b][:, qo - b * CH : qo - b * CH + wp],
                        lhsT=v_t[:, j * BS : (j + 1) * BS],
                        rhs=eh_t[:, op : op + wp],
                        start=first,
                        stop=(remaining[b] == 0),
                    )
                if rec["last"]:
                    for b in sorted(o_tiles):
                        o_sb = outpool.tile([D, CH], f16, tag="osb", name="o_sb")
                        if DRAIN_PATTERN[b % len(DRAIN_PATTERN)] == "A":
                            nc.scalar.copy(o_sb[:], o_tiles[b][:])
                        else:
                            nc.vector.tensor_copy(out=o_sb[:], in_=o_tiles[b][:])
                        nc.sync.dma_start(
                            out=ot_d[h, :, b * CH : (b + 1) * CH], in_=o_sb[:]
                        )

            for p in range(len(recs) + 4):
                if p < len(recs):
                    s0_mm1_exp(p, recs[p])
                if 1 <= p <= len(recs):
                    s2_denom_norm(p - 1, recs[p - 1])
                if p >= 4:
                    s5_mm2_drain(p - 4, recs[p - 4])

    nc.finalize()
    return nc


_CACHE = {}


def _get_program(mask):
    key = np.asarray(mask).astype(bool).tobytes()
    if key not in _CACHE:
        _CACHE[key] = _build(mask)
    return _CACHE[key]


def _shard_inputs(query, key, value):
    q = np.ascontiguousarray(query, dtype=np.float32).reshape(N_HEADS, S, D)
    k = np.ascontiguousarray(key, dtype=np.float32).reshape(N_HEADS, S, D)
    v = np.ascontiguousarray(value, dtype=np.float32).reshape(N_HEADS, S, D)
    qt = np.ascontiguousarray(q.transpose(0, 2, 1).astype(np.float16))  # (32, D, S)
    kt = np.ascontiguousarray(k.transpose(0, 2, 1).astype(np.float16))
    v16 = np.ascontiguousarray(
        v.reshape(N_HEADS, NB, BS, D).transpose(0, 2, 1, 3).astype(np.float16)
    ).reshape(N_HEADS, BS, NB * BS)
    in_maps = []
    for c in range(N_CORES):
        sl = slice(c * HPC, (c + 1) * HPC)
        in_maps.append(
            {
                "qt": np.ascontiguousarray(qt[sl]),
                "kt": np.ascontiguousarray(kt[sl]),
                "v": np.ascontiguousarray(v16[sl]),
            }
        )
    return in_maps


def _unshard_output(results):
    ot = np.concatenate([r["ot"] for r in results], axis=0)  # (32, D, S)
    out = ot.transpose(0, 2, 1).reshape(B, H, S, D)
    return np.ascontiguousarray(out, dtype=np.float32)


def kernel(query, key, value, block_mask, block_size, _trace=False):
    from concourse.bass_utils import run_bass_kernel_spmd

    assert int(block_size) == BS
    nc = _get_program(block_mask)
    in_maps = _shard_inputs(query, key, value)
    res = run_bass_kernel_spmd(nc, in_maps, core_ids=list(range(N_CORES)), trace=_trace)
    out = _unshard_output(res.results)
    if _trace:
        return out, res
    return out


# revision 47
# speedup vs baseline: 1.0116x; 1.0116x over previous
"""Block-sparse attention TRN2 kernel (8 NeuronCores, SPMD over batch*heads).

Contract: kernel(**inputs) takes FULL unsharded inputs
  query/key/value: (2, 16, 2048, 128) f32, block_mask: (16, 16) bool,
  block_size: 128
and returns the FULL (2, 16, 2048, 128) f32 output.

Math per (b, h): for each 128x128 block pair (i, j) with block_mask[i, j]:
  A_ij = softmax(Q_i K_j^T / sqrt(128)) (softmax per block row, no
  cross-block merge), O_i = sum_j A_ij V_j.

Device layout ([k, q] orientation so no on-chip transposes are needed):
  For key block j, scores for the active query blocks are packed into
  512-col (one PSUM bank) chunks: S^T = matmul(lhsT=KT[:, j], rhs=QT runs)
  in f16 (1 cyc/row at any width; scores accumulate fp32 in PSUM). exp on
  ACT (PSUM f32 -> SBUF f16). Denominators = column sums via
  matmul(lhsT=ones[128,128]), replicated across partitions in PSUM.
  reciprocal_approx_fast with f16 OUTPUT (direct _custom_dve call; the
  public wrapper only allows f32 out) so the normalize multiply runs
  f16xf16 at DVE 2x_1P rate; multiplies alternate DVE / GPSIMD to balance
  the two engines (DVE also owns every reciprocal - GPSIMD has no PSUM
  port). O^T += V_j^T.T @ Ahat^T accumulates in PSUM per 512-col output
  bank. Each head runs as two half-schedules (query rows 0-7 / 8-15) so
  only 2 output banks are live at a time, freeing PSUM for a 4-deep score
  pipeline (s 4 + d 2 + o 2 = 8 banks). Emission is software-pipelined
  (MM1/exp at p, denom/recip/mult at p-1, MM2 at p-3) so the strict-FIFO
  engine queues never head-of-line block on cross-engine dependencies.
  All heads' inputs prefetch up front; outputs drain as f16 (ACT copy +
  DMA) at the end of each half. Q^T/K^T/V packing and the final
  O^T -> O transpose happen on the host.
"""

import math

import numpy as np

B, H, S, D = 2, 16, 2048, 128
BS = 128
NB = S // BS
N_CORES = 8
N_HEADS = B * H
HPC = N_HEADS // N_CORES  # heads per core
CH = 512  # chunk columns = one PSUM bank of f32
SCALE = 1.0 / math.sqrt(float(D))

# Per-pair engine for the normalize multiply: 'D' = DVE, 'G' = GPSIMD.
# Tuned so DVE (which also owns every reciprocal) and GPSIMD balance.
MULT_PATTERN = "DGG"
# Per-bank engine for the output drain copy: 'A' = ACT, 'V' = DVE.
DRAIN_PATTERN = "AAAA"


def _plan(mask, rows):
    """Mask-derived emission plan for query rows `rows` (one output half).

    Returns a flat chunk schedule; each chunk is (used, mm1s, pieces) with
      mm1s   = (off_in_chunk, [qoff, ...], width, j); two qoffs means a
               paired single-block matmul via a 3-level access pattern.
      pieces = (q_out_col, width, off_in_chunk, j) MM2 pieces, split at
               output PSUM bank boundaries and first-touch flips.
    Partial tail chunks are merged ACROSS key blocks j (exp/denominator/
    normalize are j-agnostic; all KT/V slices are SBUF-resident), which
    cuts per-chunk op overheads on ACT/DVE by ~20%.
    """
    mask = np.asarray(mask).astype(bool)
    assert mask.shape == (NB, NB)
    rows = set(rows)
    cap = CH // BS  # blocks per chunk

    # Per-j FFD bin packing into <=cap-block bins.
    groups = []  # (j, [(i0, ln), ...]) per finalized bin, emission order
    pending = []  # [(j, item)] accumulating partial tails
    pend_fill = 0

    def flush():
        nonlocal pend_fill
        if pending:
            groups.append(list(pending))
            pending.clear()
            pend_fill = 0

    for j in range(NB):
        act = [i for i in range(NB) if i in rows and mask[i, j]]
        runs = []
        for i in act:
            if runs and runs[-1][0] + runs[-1][1] == i:
                runs[-1][1] += 1
            else:
                runs.append([i, 1])
        items = []
        for i0, ln in runs:
            while ln > cap:
                items.append((i0, cap))
                i0 += cap
                ln -= cap
            items.append((i0, ln))
        bins = []
        for i0, ln in sorted(items, key=lambda x: -x[1]):
            for b in bins:
                if b[0] + ln <= cap:
                    b[0] += ln
                    b[1].append((i0, ln))
                    break
            else:
                bins.append([ln, [(i0, ln)]])
        for fill, bitems in bins:
            if fill == cap:
                groups.append([(j, it) for it in sorted(bitems)])
            else:
                if pend_fill + fill > cap:
                    flush()
                pending.extend((j, it) for it in sorted(bitems))
                pend_fill += fill
    flush()

    # Full 512-col chunks first, partial tails last, so chunk PAIRS have at
    # most one interior gap (which the build memsets once).
    groups.sort(key=lambda g: -sum(it[1] for _, it in g))

    # Lay out each chunk and derive matmul descriptors + output pieces in
    # emission order (first-touch of an output block = overwrite; later
    # touches accumulate; a single matmul must be uniformly one or the
    # other and may not straddle an output bank).
    sched = []
    bank_counts = [0] * (S // CH)
    touched = set()
    for gitems in groups:
        byj = {}
        for j, it in gitems:
            byj.setdefault(j, []).append(it)
        placed = []  # (off, qoff, w, j)
        mm1s = []
        off = 0
        for j in sorted(byj):
            jitems = byj[j]
            longs = sorted([it for it in jitems if it[1] > 1])
            singles = sorted([it for it in jitems if it[1] == 1])
            sing_offs = []
            for i0, ln in longs + singles:
                placed.append((off, i0 * BS, ln * BS, j))
                if ln > 1:
                    mm1s.append((off, [i0 * BS], ln * BS, j))
                else:
                    sing_offs.append((off, i0 * BS))
                off += ln * BS
            for k in range(0, len(sing_offs) - 1, 2):
                mm1s.append(
                    (sing_offs[k][0], [sing_offs[k][1], sing_offs[k + 1][1]],
                     2 * BS, j)
                )
            if len(sing_offs) % 2:
                mm1s.append((sing_offs[-1][0], [sing_offs[-1][1]], BS, j))
        used = off
        pieces = []
        for o, qoff, w, j in placed:
            ib0 = qoff // BS
            nblk = w // BS
            blk = 0
            while blk < nblk:
                ib = ib0 + blk
                ft = ib not in touched
                bank = (ib * BS) // CH
                end = blk + 1
                while end < nblk:
                    ib2 = ib0 + end
                    if (ib2 not in touched) != ft or (ib2 * BS) // CH != bank:
                        break
                    end += 1
                for b2 in range(blk, end):
                    touched.add(ib0 + b2)
                qo = ib * BS
                wp = (end - blk) * BS
                pieces.append((qo, wp, o + (qo - qoff), j))
                bank_counts[bank] += 1
                blk = end
        sched.append((used, mm1s, pieces))
    empty_rows = [i for i in rows if not mask[i].any()]
    return sched, bank_counts, empty_rows


def _build(mask):
    import concourse.bass as bass
    import concourse.bacc as bacc
    import concourse.tile as tile
    from concourse import mybir
    from concourse.dve_ops import RECIP_APPROX_FAST_CONSTS, RECIPROCAL_APPROX_FAST

    f32 = mybir.dt.float32
    f16 = mybir.dt.float16
    AF = mybir.ActivationFunctionType
    RC = RECIP_APPROX_FAST_CONSTS

    # Each head runs as two half-schedules (query rows 0-7, then 8-15) so the
    # output accumulator needs only 2 PSUM banks at a time, freeing banks for
    # a deeper (bufs=3) score-pair pipeline.
    n_banks = S // CH
    rows_per_half = NB // 2
    halves = [
        _plan(mask, range(hf * rows_per_half, (hf + 1) * rows_per_half))
        for hf in range(2)
    ]

    nc = bacc.Bacc(
        "TRN2",
        target_bir_lowering=False,
        debug=False,
        enable_asserts=False,
        num_devices=N_CORES,
    )
    qt_d = nc.dram_tensor("qt", (HPC, D, S), f16, kind="ExternalInput").ap()
    kt_d = nc.dram_tensor("kt", (HPC, D, S), f16, kind="ExternalInput").ap()
    v_d = nc.dram_tensor("v", (HPC, BS, NB * BS), f16, kind="ExternalInput").ap()
    ot_d = nc.dram_tensor("ot", (HPC, D, S), f16, kind="ExternalOutput").ap()

    with tile.TileContext(nc) as tc:
        with (
            tc.tile_pool(name="heads", bufs=4) as heads,
            tc.tile_pool(name="const", bufs=1) as const,
            tc.tile_pool(name="e", bufs=6) as epool,
            tc.tile_pool(name="eh", bufs=6) as ehpool,
            tc.tile_pool(name="r", bufs=6) as rpool,
            tc.tile_pool(name="ps_d", bufs=2, space="PSUM") as ps_d,
            tc.tile_pool(name="outp", bufs=8) as outpool,
            # Split-half output (2 banks live) frees PSUM for a 4-deep
            # score pipeline: s 4 + d 2 + o 2 = 8 banks.
            tc.tile_pool(name="ps_s", bufs=4, space="PSUM") as ps_s,
            tc.tile_pool(name="ps_o", bufs=1, space="PSUM") as ps_o,
        ):
            ones_t = const.tile([BS, BS], f16)
            nc.vector.memset(ones_t[:], 1.0)
            # PE warm-up during the input-DMA window: ~5us of dummy matmuls
            # (no DMA deps) pull the HAM clock ramp (1.2 -> 2.4 GHz after
            # ~3.4us of activity) into otherwise-dead time so the first real
            # MM1s run at full rate. Output tile is never read.
            warm_t = const.tile([BS, CH], f16)
            nc.vector.memset(warm_t[:], 0.0)
            wu_ps = ps_d.tile([BS, CH], f32, name="d_ps")
            for _ in range(12):
                nc.tensor.matmul(
                    wu_ps[:], lhsT=ones_t[:], rhs=warm_t[:], start=True, stop=True
                )

            # Build the flat pair-record list (one record per chunk pair).
            recs = []
            for h_hf in range(HPC * 2):
                h, hf = divmod(h_hf, 2)
                sched, bank_counts, empty_rows = halves[hf]
                for t0 in range(len(sched)):
                    recs.append(
                        {
                            "h": h,
                            "hf": hf,
                            "chunk": sched[t0],
                            "first": t0 == 0,
                            "last": t0 + 1 >= len(sched),
                            "bank_counts": bank_counts,
                            "empty_rows": empty_rows,
                        }
                    )

            # Stage emitters. Emission order is software-pipelined so the PE
            # FIFO never head-of-line blocks: iteration p emits MM1s(p) /
            # exp(p), then denominators(p-1) / recip(p-1) / mult(p-1), then
            # MM2s(p-2) -- every PE instruction's dependencies are ~2 stages
            # old by the time the engine reaches it.
            head_tiles = {}
            half_state = {}
            # Input prefetch for all heads. Descriptor generation costs
            # ~0.65us per dma_start on the issuing engine, so the first
            # head's loads are spread across four engines to overlap the
            # generation, and later heads go on sync (drains come much
            # later, so sync has slack).
            for h in range(HPC):
                qt_t = heads.tile([D, S], f16, tag="qt", name="qt_t")
                kt_t = heads.tile([D, S], f16, tag="kt", name="kt_t")
                v_t = heads.tile([BS, NB * BS], f16, tag="v", name="v_t")
                if h == 0:
                    for q4 in range(4):
                        sl = slice(q4 * S // 4, (q4 + 1) * S // 4)
                        nc.sync.dma_start(out=kt_t[:, sl], in_=kt_d[h, :, sl])
                    for q2 in range(2):
                        sl = slice(q2 * S // 2, (q2 + 1) * S // 2)
                        nc.scalar.dma_start(out=qt_t[:, sl], in_=qt_d[h, :, sl])
                    nc.gpsimd.dma_start(out=v_t[:], in_=v_d[h])
                else:
                    nc.sync.dma_start(out=kt_t[:], in_=kt_d[h])
                    nc.sync.dma_start(out=qt_t[:], in_=qt_d[h])
                    nc.sync.dma_start(out=v_t[:], in_=v_d[h])
                head_tiles[h] = (qt_t, kt_t, v_t)

            def s0_mm1_exp(p, rec):
                h, hf = rec["h"], rec["hf"]
                qt_t, kt_t, v_t = head_tiles[h]
                if rec["first"]:
                    hbank = hf * (n_banks // 2)
                    o_tiles = {
                        b: ps_o.tile([D, CH], f32, name=f"ob{b % 2}", tag=f"ot{b % 2}")
                        for b in range(hbank, hbank + n_banks // 2)
                    }
                    for i in rec["empty_rows"]:
                        b, c = divmod(i * BS, CH)
                        nc.vector.memset(o_tiles[b][:, c : c + BS], 0.0)
                    half_state[(h, hf)] = {
                        "o_tiles": o_tiles,
                        "remaining": list(rec["bank_counts"]),
                        "started": set(),
                    }
                used, mm1s, pieces = rec["chunk"]
                s_ps = ps_s.tile([BS, CH], f32, name="s_ps")
                for idx, (off, qoffs, w, j) in enumerate(mm1s):
                    if len(qoffs) == 2:
                        base = qt_t[:, qoffs[0] : qoffs[0] + BS]
                        rhs = bass.AP(
                            tensor=base.tensor,
                            offset=base.offset,
                            ap=[
                                base.ap[0],
                                [qoffs[1] - qoffs[0], 2],
                                [1, BS],
                            ],
                        )
                    else:
                        rhs = qt_t[:, qoffs[0] : qoffs[0] + w]
                    nc.tensor.matmul(
                        s_ps[:, off : off + w],
                        lhsT=kt_t[:, j * BS : (j + 1) * BS],
                        rhs=rhs,
                        start=(idx == 0),
                        stop=(idx == len(mm1s) - 1),
                    )
                e_t = epool.tile([BS, CH], f16, name="e_t")
                nc.scalar.activation(
                    e_t[:, :used], s_ps[:, :used], AF.Exp, scale=SCALE
                )
                rec["s_ps"], rec["e_t"] = s_ps, e_t

            def s2_denom_norm(p, rec):
                e_t = rec["e_t"]
                used = rec["chunk"][0]
                d_ps = ps_d.tile([BS, CH], f32, name="d_ps")
                nc.tensor.matmul(
                    d_ps[:, :used],
                    lhsT=ones_t[:],
                    rhs=e_t[:, :used],
                    start=True,
                    stop=True,
                )
                # reciprocal_approx_fast with f16 output (direct custom-DVE
                # call; ~51 ULP in f32, then f16 rounding) so the multiply
                # below gets DVE 2x_1P rate.
                r_t = rpool.tile([BS, CH], f16, name="r_t")
                nc.vector._custom_dve(
                    RECIPROCAL_APPROX_FAST,
                    out=r_t[:, :used],
                    in0=d_ps[:, :used],
                    s0=RC["s0"],
                    s1=RC["s1"],
                    imm2=RC["imm2"],
                )
                eh_t = ehpool.tile([BS, CH], f16, name="eh_t")
                mult_on = MULT_PATTERN[p % len(MULT_PATTERN)]
                mult_eng = nc.gpsimd if mult_on == "G" else nc.vector
                mult_eng.tensor_tensor(
                    out=eh_t[:, :used],
                    in0=e_t[:, :used],
                    in1=r_t[:, :used],
                    op=mybir.AluOpType.mult,
                )
                rec["eh_t"] = eh_t

            def s5_mm2_drain(p, rec):
                h, hf = rec["h"], rec["hf"]
                st = half_state[(h, hf)]
                o_tiles, remaining, started = (
                    st["o_tiles"],
                    st["remaining"],
                    st["started"],
                )
                _, _, v_t = head_tiles[h]
                eh_t = rec["eh_t"]
                for qo, wp, op, j in rec["chunk"][2]:
                    b = qo // CH
                    first = b not in started
                    started.add(b)
                    remaining[b] -= 1
                    nc.tensor.matmul(
                        o_tiles[b][:, qo - b * CH : qo - b * CH + wp],
                        lhsT=v_t[:, j * BS : (j + 1) * BS],
                        rhs=eh_t[:, op : op + wp],
                        start=first,
                        stop=(remaining[b] == 0),
                    )
                if rec["last"]:
                    for b in sorted(o_tiles):
                        o_sb = outpool.tile([D, CH], f16, tag="osb", name="o_sb")
                        if DRAIN_PATTERN[b % len(DRAIN_PATTERN)] == "A":
                            nc.scalar.copy(o_sb[:], o_tiles[b][:])
                        else:
                            nc.vector.tensor_copy(out=o_sb[:], in_=o_tiles[b][:])
                        nc.sync.dma_start(
                            out=ot_d[h, :, b * CH : (b + 1) * CH], in_=o_sb[:]
                        )

            for p in range(len(recs) + 3):
                if p < len(recs):
                    s0_mm1_exp(p, recs[p])
                if 1 <= p <= len(recs):
                    s2_denom_norm(p - 1, recs[p - 1])
                if p >= 3:
                    s5_mm2_drain(p - 3, recs[p - 3])

    nc.finalize()
    return nc


_CACHE = {}


def _get_program(mask):
    key = np.asarray(mask).astype(bool).tobytes()
    if key not in _CACHE:
        _CACHE[key] = _build(mask)
    return _CACHE[key]


def _shard_inputs(query, key, value):
    q = np.ascontiguousarray(query, dtype=np.float32).reshape(N_HEADS, S, D)
    k = np.ascontiguousarray(key, dtype=np.float32).reshape(N_HEADS, S, D)
    v = np.ascontiguousarray(value, dtype=np.float32).reshape(N_HEADS, S, D)
    qt = np.ascontiguousarray(q.transpose(0, 2, 1).astype(np.float16))  # (32, D, S)
    kt = np.ascontiguousarray(k.transpose(0, 2, 1).astype(np.float16))
    v16 = np.ascontiguousarray(
        v.reshape(N_HEADS, NB, BS, D).transpose(0, 2, 1, 3).astype(np.float16)
    ).reshape(N_HEADS, BS, NB * BS)
    in_maps = []
    for c in range(N_CORES):
        sl = slice(c * HPC, (c + 1) * HPC)
        in_maps.append(
            {
                "qt": np.ascontiguousarray(qt[sl]),
                "kt": np.ascontiguousarray(kt[sl]),
                "v": np.ascontiguousarray(v16[sl]),
            }
        )
    return in_maps


def _unshard_output(results):
    ot = np.concatenate([r["ot"] for r in results], axis=0)  # (32, D, S)
    out = ot.transpose(0, 2, 1).reshape(B, H, S, D)
    return np.ascontiguousarray(out, dtype=np.float32)


def kernel(query, key, value, block_mask, block_size, _trace=False):
    from concourse.bass_utils import run_bass_kernel_spmd

    assert int(block_size) == BS
    nc = _get_program(block_mask)
    in_maps = _shard_inputs(query, key, value)
    res = run_bass_kernel_spmd(nc, in_maps, core_ids=list(range(N_CORES)), trace=_trace)
    out = _unshard_output(res.results)
    if _trace:
        return out, res
    return out
